# revision 1
# baseline (speedup 1.0000x reference)
"""Trainium2 Bass kernel for nn_Embedding2Score (segment_reduce).

Reference computation:
    v_n  = x[last_idx]                               [B, H]
    h    = sigmoid((v_n @ W1^T + b1)[batch] + x @ W2^T + b2)
    alpha= h @ q^T + q_b                             [N, 1]
    s_g  = segment_sum(alpha * x, batch)             [B, H]
    s_h  = [v_n, s_g] @ W3^T + b3                    [B, H]
    z    = s_h @ emb[1:]^T                           [B, V-1]

Sharding (8 cores): phase 1 is data-parallel over segments (256 sorted
sessions' worth of nodes per core); phase 2 is data-parallel over vocab
columns (12800 emb rows per core, all 2048 segments). The tiny s_h
[2048,128] is gathered on the host between the two SPMD launches.

One SPMD program serves all 8 cores: everything data-dependent (segment
ids, gather indices) is passed as per-core input tensors; segment
one-hot masks are built on-chip with iota + is_equal against narrow
windows that are an affine function of the chunk index (validated on
the host per call, with a windowless fallback program). The per-node
session bias is reconstructed with a PE matmul against the transposed
mask (no per-chunk DMA gathers), and the final big matmul runs as a
bf16 hi/lo 3-matmul decomposition (exact products, fp32 accumulate).
"""
import numpy as np
import ml_dtypes

import concourse.bass as bass
import concourse.tile as tile
import concourse.mybir as mybir
from concourse import bacc
from concourse import bass_utils
from concourse.masks import make_identity

F32 = mybir.dt.float32
BF16 = mybir.dt.bfloat16
I32 = mybir.dt.int32

N_NODES = 102400
B_SEG = 2048
H = 128
VOCAB = 100000
NCORES = 8
SEG_C = B_SEG // NCORES          # 256 segments per core
VSHARD = 12544                   # vocab columns per core (padded)
NTILE = 448                      # phase-2 matmul free dim
NCHUNK = 1792                    # phase-2 staging width (4 matmuls)
MW = 128                         # bias-path mask window (64-grid aligned)
SW = 40                          # s_g-path mask window (unquantized)


def _bc(ap, ins_axis, n):
    """Insert a 0-step broadcast dim into an AP at ins_axis."""
    l = list(ap.ap)
    l.insert(ins_axis, [0, n])
    return bass.AP(tensor=ap.tensor, offset=ap.offset, ap=l)


def affine_windows(nmax, mask_w, grid=1):
    """Core-uniform per-chunk segment-window starts (affine in chunk idx)."""
    nt = nmax // 128
    return [min(max(0, grid * round((round(n * SEG_C / nt) - mask_w // 2) / grid)),
                SEG_C - mask_w) for n in range(nt)]


def windows_ok(blf_list, nmax, mask_w, windows):
    nt = nmax // 128
    for blf in blf_list:
        bl = blf.T.reshape(-1)
        for n in range(nt):
            lo = int(bl[n * 128:(n + 1) * 128].min())
            hi = int(bl[n * 128:(n + 1) * 128].max())
            if lo < windows[n] or hi >= windows[n] + mask_w:
                return False
    return True


def _phase1_common(nc, nmax):
    """Declare phase-1 dram tensors (shared by both builds)."""
    nt = nmax // 128
    d = {}
    d["x"] = nc.dram_tensor("x", [nmax, H], F32, kind="ExternalInput")
    d["xT"] = nc.dram_tensor("xT", [H, nmax], F32, kind="ExternalInput")
    d["blf"] = nc.dram_tensor("blf", [128, nt], F32, kind="ExternalInput")
    d["bli"] = nc.dram_tensor("bli", [128, nt], I32, kind="ExternalInput")
    d["lastloc"] = nc.dram_tensor("lastloc", [128, 2], I32, kind="ExternalInput")
    d["W1T"] = nc.dram_tensor("W1T", [H, H], F32, kind="ExternalInput")
    d["W2T"] = nc.dram_tensor("W2T", [H, H], F32, kind="ExternalInput")
    d["W3aT"] = nc.dram_tensor("W3aT", [H, H], F32, kind="ExternalInput")
    d["W3bT"] = nc.dram_tensor("W3bT", [H, H], F32, kind="ExternalInput")
    d["b12"] = nc.dram_tensor("b12", [1, H], F32, kind="ExternalInput")
    d["w3brow"] = nc.dram_tensor("w3brow", [1, H], F32, kind="ExternalInput")
    d["qrow"] = nc.dram_tensor("qrow", [1, H], F32, kind="ExternalInput")
    d["qb"] = nc.dram_tensor("qb", [1, 1], F32, kind="ExternalInput")
    d["s_h"] = nc.dram_tensor("s_h", [SEG_C, H], F32, kind="ExternalOutput")
    return d


def _build_phase1(nmax, windows, swin):
    """Windowed 'bcmm' phase 1: bias via PE mask-transpose matmul."""
    nt = nmax // 128
    ng = nmax // 512
    nc = bacc.Bacc("TRN2")
    d = _phase1_common(nc, nmax)

    with tile.TileContext(nc) as tc:
        with (
            tc.tile_pool(name="const", bufs=1) as const,
            tc.tile_pool(name="xs", bufs=3) as xs,
            tc.tile_pool(name="work", bufs=3) as work,
            tc.tile_pool(name="ps", bufs=1, space="PSUM") as ps,
            tc.tile_pool(name="psw", bufs=2, space="PSUM") as psw,
            tc.tile_pool(name="pst", bufs=5, space="PSUM") as pst,
            tc.tile_pool(name="sgp", bufs=1, space="PSUM") as sgp,
        ):
            ident = const.tile([128, 128], F32)
            make_identity(nc, ident[:])
            ident_bf = const.tile([128, 128], BF16)
            make_identity(nc, ident_bf[:])
            iota_i = const.tile([128, SEG_C], I32)
            nc.gpsimd.iota(iota_i[:], pattern=[[1, SEG_C]], base=0,
                           channel_multiplier=0)
            iota_f = const.tile([128, SEG_C], F32)
            nc.vector.tensor_copy(iota_f[:], iota_i[:])
            ones1 = const.tile([1, 128], F32)
            nc.vector.memset(ones1[:], 1.0)
            w1t = const.tile([H, H], F32)
            nc.sync.dma_start(w1t[:], d["W1T"][:, :])
            w2t = const.tile([H, H], F32)
            nc.sync.dma_start(w2t[:], d["W2T"][:, :])
            w3at = const.tile([H, H], F32)
            nc.sync.dma_start(w3at[:], d["W3aT"][:, :])
            w3bt = const.tile([H, H], F32)
            nc.sync.dma_start(w3bt[:], d["W3bT"][:, :])
            b12 = const.tile([1, H], F32)
            nc.sync.dma_start(b12[:], d["b12"][:, :])
            w3brow = const.tile([1, H], F32)
            nc.sync.dma_start(w3brow[:], d["w3brow"][:, :])
            qrow = const.tile([1, H], F32)
            nc.sync.dma_start(qrow[:], d["qrow"][:, :])
            qb = const.tile([128, 1], F32)
            nc.sync.dma_start(qb[:], d["qb"][:, :].partition_broadcast(128))
            blf = const.tile([128, nt], F32)
            nc.sync.dma_start(blf[:], d["blf"][:, :])
            lastloc = const.tile([128, 2], I32)
            nc.sync.dma_start(lastloc[:], d["lastloc"][:, :])

            qps = pst.tile([128, 128], F32, tag="mt")
            nc.tensor.matmul(qps[:], ones1[:], qrow[:], start=True, stop=True)
            q_bcast = const.tile([128, 128], F32)
            nc.vector.tensor_copy(q_bcast[:], qps[:])

            vn = const.tile([128, 2, H], F32)
            vnT = const.tile([H, SEG_C], F32)
            # w1b2 blocks: [:,0]=segs 0:128, [:,1]=128:256, [:,2]=64:192,
            # each split into bf16 hi/lo for 1-cyc/row bias matmuls
            w1b2_hi = const.tile([128, 3, H], BF16)
            w1b2_lo = const.tile([128, 3, H], BF16)
            w1b2_tmp = const.tile([128, H], F32)
            for t in range(2):
                nc.gpsimd.indirect_dma_start(
                    out=vn[:, t, :], out_offset=None, in_=d["x"][:, :],
                    in_offset=bass.IndirectOffsetOnAxis(
                        ap=lastloc[:, t:t + 1], axis=0))
                tp = pst.tile([128, 128], F32, tag="mt")
                nc.tensor.transpose(tp[:], vn[:, t, :], ident[:])
                nc.vector.tensor_copy(vnT[:, t * 128:(t + 1) * 128], tp[:])
            for t, s0 in ((0, 0), (1, 128), (2, 64)):
                pw = pst.tile([128, 128], F32, tag="mt")
                nc.tensor.matmul(pw[:], ones1[:], b12[:], start=True, stop=False)
                nc.tensor.matmul(pw[:], vnT[:, s0:s0 + 128], w1t[:],
                                 start=False, stop=True)
                nc.scalar.copy(w1b2_hi[:, t, :], pw[:])
                nc.vector.tensor_tensor(w1b2_tmp[:], pw[:], w1b2_hi[:, t, :],
                                        op=mybir.AluOpType.subtract)
                nc.vector.tensor_copy(w1b2_lo[:, t, :], w1b2_tmp[:])

            sg_ps = sgp.tile([128, SEG_C], F32)
            zrow = const.tile([1, SEG_C], F32)
            nc.vector.memset(zrow[:], 0.0)
            nc.tensor.matmul(sg_ps[:], ones1[:], zrow[:],
                             start=True, stop=True, skip_group_check=True)

            for g in range(ng):
                x_sb = xs.tile([128, 4, H], F32)
                nc.sync.dma_start(
                    x_sb[:],
                    d["x"][g * 512:(g + 1) * 512, :].rearrange(
                        "(c p) h -> p c h", p=128))
                xT_sb = xs.tile([H, 512], F32)
                nc.sync.dma_start(xT_sb[:], d["xT"][:, g * 512:(g + 1) * 512])

                p1g = psw.tile([128, 512], F32, tag="p1")
                pmask = work.tile([128, 4, MW], BF16, tag="pm")
                for c in range(4):
                    n = g * 4 + c
                    st = windows[n]
                    nc.tensor.matmul(p1g[:, c * 128:(c + 1) * 128],
                                     xT_sb[:, c * 128:(c + 1) * 128],
                                     w2t[:], start=True, stop=False,
                                     skip_group_check=True)
                    nc.vector.tensor_scalar(
                        pmask[:, c, :], iota_f[:, st:st + MW],
                        blf[:, n:n + 1], None, mybir.AluOpType.is_equal)
                    tpm = pst.tile([MW, 128], BF16, tag="mt")
                    nc.tensor.transpose(tpm[:], pmask[:, c, :], ident_bf[:])
                    mT = work.tile([MW, 128], BF16, tag="mTs")
                    nc.scalar.copy(mT[:], tpm[:])
                    blk = {0: 0, 64: 2, 128: 1}[st]
                    nc.tensor.matmul(
                        p1g[:, c * 128:(c + 1) * 128], mT[:],
                        w1b2_hi[:, blk, :],
                        start=False, stop=False, skip_group_check=True)
                    nc.tensor.matmul(
                        p1g[:, c * 128:(c + 1) * 128], mT[:],
                        w1b2_lo[:, blk, :],
                        start=False, stop=True, skip_group_check=True)
                hsb = work.tile([128, 4, H], F32)
                nc.scalar.activation(hsb[:].rearrange("p a b -> p (a b)"),
                                     p1g[:],
                                     mybir.ActivationFunctionType.Sigmoid)
                hq = work.tile([128, 4, H], F32)
                nc.vector.tensor_tensor(hq[:], hsb[:], _bc(q_bcast[:], 1, 4),
                                        op=mybir.AluOpType.mult)
                araw = work.tile([128, 4], F32)
                nc.vector.reduce_sum(araw[:], hq[:], axis=mybir.AxisListType.X)
                alpha = work.tile([128, 4], F32)
                nc.vector.tensor_tensor(alpha[:], araw[:],
                                        qb[:].to_broadcast([128, 4]),
                                        op=mybir.AluOpType.add)
                mask = work.tile([128, 4, SW], F32, tag="ma")
                for c in range(4):
                    n = g * 4 + c
                    st = swin[n]
                    nc.vector.tensor_scalar(
                        mask[:, c, :], iota_f[:, st:st + SW],
                        blf[:, n:n + 1], alpha[:, c:c + 1],
                        mybir.AluOpType.is_equal, mybir.AluOpType.mult)
                    nc.tensor.matmul(
                        sg_ps[:, st:st + SW], x_sb[:, c, :], mask[:, c, :],
                        start=False, stop=(n == nt - 1),
                        skip_group_check=True)

            sgT = const.tile([H, SEG_C], F32)
            nc.vector.tensor_copy(sgT[:], sg_ps[:])
            shs = const.tile([128, 2, H], F32)
            for t in range(2):
                psh = pst.tile([128, 128], F32, tag="mt")
                nc.tensor.matmul(psh[:], ones1[:], w3brow[:], start=True,
                                 stop=False)
                nc.tensor.matmul(psh[:], vnT[:, t * 128:(t + 1) * 128],
                                 w3at[:], start=False, stop=False)
                nc.tensor.matmul(psh[:], sgT[:, t * 128:(t + 1) * 128],
                                 w3bt[:], start=False, stop=True)
                nc.vector.tensor_copy(shs[:, t, :], psh[:])
                nc.sync.dma_start(d["s_h"][t * 128:(t + 1) * 128, :],
                                  shs[:, t, :])
    nc.compile()
    return nc


def _build_phase1_fallback(nmax):
    """Full-width-mask phase 1 with per-chunk bias gathers (no windows)."""
    nt = nmax // 128
    ng = nmax // 512
    nc = bacc.Bacc("TRN2")
    d = _phase1_common(nc, nmax)
    w1b2_d = nc.dram_tensor("w1b2_scratch", [SEG_C, H], F32)

    with tile.TileContext(nc) as tc:
        with (
            tc.tile_pool(name="const", bufs=1) as const,
            tc.tile_pool(name="xs", bufs=3) as xs,
            tc.tile_pool(name="work", bufs=3) as work,
            tc.tile_pool(name="ps", bufs=2, space="PSUM") as ps,
            tc.tile_pool(name="psw", bufs=3, space="PSUM") as psw,
            tc.tile_pool(name="sgp", bufs=1, space="PSUM") as sgp,
        ):
            ident = const.tile([128, 128], F32)
            make_identity(nc, ident[:])
            iota_i = const.tile([128, SEG_C], I32)
            nc.gpsimd.iota(iota_i[:], pattern=[[1, SEG_C]], base=0,
                           channel_multiplier=0)
            iota_f = const.tile([128, SEG_C], F32)
            nc.vector.tensor_copy(iota_f[:], iota_i[:])
            ones1 = const.tile([1, 128], F32)
            nc.vector.memset(ones1[:], 1.0)
            w1t = const.tile([H, H], F32)
            nc.sync.dma_start(w1t[:], d["W1T"][:, :])
            w2t = const.tile([H, H], F32)
            nc.sync.dma_start(w2t[:], d["W2T"][:, :])
            w3at = const.tile([H, H], F32)
            nc.sync.dma_start(w3at[:], d["W3aT"][:, :])
            w3bt = const.tile([H, H], F32)
            nc.sync.dma_start(w3bt[:], d["W3bT"][:, :])
            b12 = const.tile([1, H], F32)
            nc.sync.dma_start(b12[:], d["b12"][:, :])
            w3brow = const.tile([1, H], F32)
            nc.sync.dma_start(w3brow[:], d["w3brow"][:, :])
            qrow = const.tile([1, H], F32)
            nc.sync.dma_start(qrow[:], d["qrow"][:, :])
            qb = const.tile([128, 1], F32)
            nc.sync.dma_start(qb[:], d["qb"][:, :].partition_broadcast(128))
            blf = const.tile([128, nt], F32)
            nc.sync.dma_start(blf[:], d["blf"][:, :])
            bli = const.tile([128, nt], I32)
            nc.sync.dma_start(bli[:], d["bli"][:, :])
            lastloc = const.tile([128, 2], I32)
            nc.sync.dma_start(lastloc[:], d["lastloc"][:, :])

            qps = ps.tile([128, 128], F32, tag="mm")
            nc.tensor.matmul(qps[:], ones1[:], qrow[:], start=True, stop=True)
            q_bcast = const.tile([128, 128], F32)
            nc.vector.tensor_copy(q_bcast[:], qps[:])

            vn = const.tile([128, 2, H], F32)
            vnT = const.tile([H, SEG_C], F32)
            w1b2 = const.tile([128, 2, H], F32)
            for t in range(2):
                nc.gpsimd.indirect_dma_start(
                    out=vn[:, t, :], out_offset=None, in_=d["x"][:, :],
                    in_offset=bass.IndirectOffsetOnAxis(
                        ap=lastloc[:, t:t + 1], axis=0))
                tp = ps.tile([128, 128], F32, tag="mm")
                nc.tensor.transpose(tp[:], vn[:, t, :], ident[:])
                nc.vector.tensor_copy(vnT[:, t * 128:(t + 1) * 128], tp[:])
                pw = ps.tile([128, 128], F32, tag="mm")
                nc.tensor.matmul(pw[:], ones1[:], b12[:], start=True, stop=False)
                nc.tensor.matmul(pw[:], vnT[:, t * 128:(t + 1) * 128], w1t[:],
                                 start=False, stop=True)
                nc.vector.tensor_copy(w1b2[:, t, :], pw[:])
                nc.sync.dma_start(w1b2_d[t * 128:(t + 1) * 128, :], w1b2[:, t, :])

            sg_ps = sgp.tile([128, SEG_C], F32)
            for g in range(ng):
                x_sb = xs.tile([128, 4, H], F32)
                nc.sync.dma_start(
                    x_sb[:],
                    d["x"][g * 512:(g + 1) * 512, :].rearrange(
                        "(c p) h -> p c h", p=128))
                xT_sb = xs.tile([H, 512], F32)
                nc.sync.dma_start(xT_sb[:], d["xT"][:, g * 512:(g + 1) * 512])

                p1g = psw.tile([128, 512], F32, tag="p1")
                for c in range(4):
                    nc.tensor.matmul(p1g[:, c * 128:(c + 1) * 128],
                                     xT_sb[:, c * 128:(c + 1) * 128],
                                     w2t[:], start=True, stop=True)
                hpre = work.tile([128, 4, H], F32)
                hpre_flat = hpre[:].rearrange("p a b -> p (a b)")
                nc.scalar.copy(hpre_flat, p1g[:])
                for c in range(4):
                    nc.gpsimd.indirect_dma_start(
                        out=hpre[:, c, :], out_offset=None, in_=w1b2_d[:, :],
                        in_offset=bass.IndirectOffsetOnAxis(
                            ap=bli[:, 4 * g + c:4 * g + c + 1], axis=0),
                        compute_op=mybir.AluOpType.add)
                hsb = work.tile([128, 4, H], F32)
                nc.scalar.activation(hsb[:].rearrange("p a b -> p (a b)"),
                                     hpre_flat,
                                     mybir.ActivationFunctionType.Sigmoid)
                hq = work.tile([128, 4, H], F32)
                nc.vector.tensor_tensor(hq[:], hsb[:], _bc(q_bcast[:], 1, 4),
                                        op=mybir.AluOpType.mult)
                araw = work.tile([128, 4], F32)
                nc.vector.reduce_sum(araw[:], hq[:], axis=mybir.AxisListType.X)
                alpha = work.tile([128, 4], F32)
                nc.vector.tensor_tensor(alpha[:], araw[:],
                                        qb[:].to_broadcast([128, 4]),
                                        op=mybir.AluOpType.add)
                mask = work.tile([128, 4, SEG_C], F32, tag="ma")
                for c in range(4):
                    n = g * 4 + c
                    nc.vector.tensor_scalar(
                        mask[:, c, :], iota_f[:],
                        blf[:, n:n + 1], alpha[:, c:c + 1],
                        mybir.AluOpType.is_equal, mybir.AluOpType.mult)
                    nc.tensor.matmul(sg_ps[:], x_sb[:, c, :], mask[:, c, :],
                                     start=(n == 0), stop=(n == nt - 1))

            sgT = const.tile([H, SEG_C], F32)
            nc.vector.tensor_copy(sgT[:], sg_ps[:])
            shs = const.tile([128, 2, H], F32)
            for t in range(2):
                psh = ps.tile([128, 128], F32, tag="mm")
                nc.tensor.matmul(psh[:], ones1[:], w3brow[:], start=True,
                                 stop=False)
                nc.tensor.matmul(psh[:], vnT[:, t * 128:(t + 1) * 128],
                                 w3at[:], start=False, stop=False)
                nc.tensor.matmul(psh[:], sgT[:, t * 128:(t + 1) * 128],
                                 w3bt[:], start=False, stop=True)
                nc.vector.tensor_copy(shs[:, t, :], psh[:])
                nc.sync.dma_start(d["s_h"][t * 128:(t + 1) * 128, :],
                                  shs[:, t, :])
    nc.compile()
    return nc


def _build_phase2():
    """Per-core: z shard [B_SEG, VSHARD] = s_h @ ET_shard via bf16 hi/lo."""
    nc = bacc.Bacc("TRN2")
    sh_hi_d = nc.dram_tensor("shT_hi", [H, B_SEG], BF16, kind="ExternalInput")
    sh_lo_d = nc.dram_tensor("shT_lo", [H, B_SEG], BF16, kind="ExternalInput")
    et_hi_d = nc.dram_tensor("ET_hi", [H, VSHARD], BF16, kind="ExternalInput")
    et_lo_d = nc.dram_tensor("ET_lo", [H, VSHARD], BF16, kind="ExternalInput")
    z_d = nc.dram_tensor("z", [B_SEG, VSHARD], F32, kind="ExternalOutput")
    nch = VSHARD // NCHUNK
    ntm = NCHUNK // NTILE
    with tile.TileContext(nc) as tc:
        with (
            tc.tile_pool(name="const", bufs=1) as const,
            tc.tile_pool(name="stage", bufs=4) as stage,
            tc.tile_pool(name="ps", bufs=8, space="PSUM") as ps,
        ):
            sh_hi = const.tile([H, B_SEG], BF16)
            nc.sync.dma_start(sh_hi[:], sh_hi_d[:, :])
            sh_lo = const.tile([H, B_SEG], BF16)
            nc.sync.dma_start(sh_lo[:], sh_lo_d[:, :])
            eth, etl = [], []
            for i in range(nch):
                a = const.tile([H, NCHUNK], BF16, tag=f"eth{i}")
                nc.sync.dma_start(a[:], et_hi_d[:, i * NCHUNK:(i + 1) * NCHUNK])
                eth.append(a)
                b = const.tile([H, NCHUNK], BF16, tag=f"etl{i}")
                nc.sync.dma_start(b[:], et_lo_d[:, i * NCHUNK:(i + 1) * NCHUNK])
                etl.append(b)
            k = 0
            for m in range(B_SEG // 128):
                ms = slice(m * 128, (m + 1) * 128)
                for i in range(nch):
                    stg = stage.tile([128, NCHUNK], F32)
                    for j in range(ntm):
                        js = slice(j * NTILE, (j + 1) * NTILE)
                        pz = ps.tile([128, NTILE], F32)
                        nc.tensor.matmul(pz[:], sh_hi[:, ms], eth[i][:, js],
                                         start=True, stop=False)
                        nc.tensor.matmul(pz[:], sh_hi[:, ms], etl[i][:, js],
                                         start=False, stop=False)
                        nc.tensor.matmul(pz[:], sh_lo[:, ms], eth[i][:, js],
                                         start=False, stop=True)
                        dst = stg[:, js]
                        if k % 2 == 0:
                            nc.vector.tensor_copy(dst, pz[:])
                        else:
                            nc.scalar.copy(dst, pz[:])
                        k += 1
                    nc.sync.dma_start(
                        z_d[m * 128:(m + 1) * 128,
                            i * NCHUNK:(i + 1) * NCHUNK], stg[:])
    nc.compile()
    return nc


def _build_merged(nmax, windows, swin):
    nt = nmax // 128
    ng = nmax // 512
    nc = bacc.Bacc("TRN2", num_devices=8)
    d = {}
    d["x"] = nc.dram_tensor("x", [nmax, H], F32, kind="ExternalInput")
    d["xT"] = nc.dram_tensor("xT", [H, nmax], F32, kind="ExternalInput")
    d["blf"] = nc.dram_tensor("blf", [128, nt], F32, kind="ExternalInput")
    d["lastloc"] = nc.dram_tensor("lastloc", [128, 2], I32, kind="ExternalInput")
    d["W1T"] = nc.dram_tensor("W1T", [H, H], F32, kind="ExternalInput")
    d["W2T"] = nc.dram_tensor("W2T", [H, H], F32, kind="ExternalInput")
    d["W3aT"] = nc.dram_tensor("W3aT", [H, H], F32, kind="ExternalInput")
    d["W3bT"] = nc.dram_tensor("W3bT", [H, H], F32, kind="ExternalInput")
    d["b12"] = nc.dram_tensor("b12", [1, H], F32, kind="ExternalInput")
    d["w3brow"] = nc.dram_tensor("w3brow", [1, H], F32, kind="ExternalInput")
    d["qrow"] = nc.dram_tensor("qrow", [1, H], F32, kind="ExternalInput")
    d["qb"] = nc.dram_tensor("qb", [1, 1], F32, kind="ExternalInput")
    et_hi_d = nc.dram_tensor("ET_hi", [H, VSHARD], BF16, kind="ExternalInput")
    et_lo_d = nc.dram_tensor("ET_lo", [H, VSHARD], BF16, kind="ExternalInput")
    z_d = nc.dram_tensor("z", [B_SEG, VSHARD], F32, kind="ExternalOutput")
    cc_in = nc.dram_tensor("cc_in", [SEG_C, H], F32)
    cc_out = nc.dram_tensor("cc_out", [B_SEG, H], F32, addr_space="Shared")

    nch = VSHARD // NCHUNK
    ntm = NCHUNK // NTILE
    with tile.TileContext(nc) as tc:
        with (
            tc.tile_pool(name="const", bufs=1) as const,
            tc.tile_pool(name="xs", bufs=3) as xs,
            tc.tile_pool(name="work", bufs=3) as work,
            tc.tile_pool(name="psw", bufs=2, space="PSUM") as psw,
            tc.tile_pool(name="pst", bufs=5, space="PSUM") as pst,
            tc.tile_pool(name="sgp", bufs=1, space="PSUM") as sgp,
            tc.tile_pool(name="stage", bufs=4) as stage,
        ):
            ident = const.tile([128, 128], F32)
            make_identity(nc, ident[:])
            ident_bf = const.tile([128, 128], BF16)
            make_identity(nc, ident_bf[:])
            iota_i = const.tile([128, SEG_C], I32)
            nc.gpsimd.iota(iota_i[:], pattern=[[1, SEG_C]], base=0,
                           channel_multiplier=0)
            iota_f = const.tile([128, SEG_C], F32)
            nc.vector.tensor_copy(iota_f[:], iota_i[:])
            ones1 = const.tile([1, 128], F32)
            nc.vector.memset(ones1[:], 1.0)
            w1t = const.tile([H, H], F32)
            nc.sync.dma_start(w1t[:], d["W1T"][:, :])
            w2t = const.tile([H, H], F32)
            nc.sync.dma_start(w2t[:], d["W2T"][:, :])
            w3at = const.tile([H, H], F32)
            nc.sync.dma_start(w3at[:], d["W3aT"][:, :])
            w3bt = const.tile([H, H], F32)
            nc.sync.dma_start(w3bt[:], d["W3bT"][:, :])
            b12 = const.tile([1, H], F32)
            nc.sync.dma_start(b12[:], d["b12"][:, :])
            w3brow = const.tile([1, H], F32)
            nc.sync.dma_start(w3brow[:], d["w3brow"][:, :])
            qrow = const.tile([1, H], F32)
            nc.sync.dma_start(qrow[:], d["qrow"][:, :])
            qb = const.tile([128, 1], F32)
            nc.sync.dma_start(qb[:], d["qb"][:, :].partition_broadcast(128))
            blf = const.tile([128, nt], F32)
            nc.sync.dma_start(blf[:], d["blf"][:, :])
            lastloc = const.tile([128, 2], I32)
            nc.sync.dma_start(lastloc[:], d["lastloc"][:, :])

            qps = pst.tile([128, 128], F32, tag="mt")
            nc.tensor.matmul(qps[:], ones1[:], qrow[:], start=True, stop=True)
            q_bcast = const.tile([128, 128], F32)
            nc.vector.tensor_copy(q_bcast[:], qps[:])

            vn = const.tile([128, 2, H], F32)
            vnT = const.tile([H, SEG_C], F32)
            # w1b2 blocks: [:,0]=segs 0:128, [:,1]=128:256, [:,2]=64:192,
            # each split into bf16 hi/lo for 1-cyc/row bias matmuls
            w1b2_hi = const.tile([128, 3, H], BF16)
            w1b2_lo = const.tile([128, 3, H], BF16)
            w1b2_tmp = const.tile([128, H], F32)
            for t in range(2):
                nc.gpsimd.indirect_dma_start(
                    out=vn[:, t, :], out_offset=None, in_=d["x"][:, :],
                    in_offset=bass.IndirectOffsetOnAxis(
                        ap=lastloc[:, t:t + 1], axis=0))
                tp = pst.tile([128, 128], F32, tag="mt")
                nc.tensor.transpose(tp[:], vn[:, t, :], ident[:])
                nc.vector.tensor_copy(vnT[:, t * 128:(t + 1) * 128], tp[:])
            for t, s0 in ((0, 0), (1, 128), (2, 64)):
                pw = pst.tile([128, 128], F32, tag="mt")
                nc.tensor.matmul(pw[:], ones1[:], b12[:], start=True, stop=False)
                nc.tensor.matmul(pw[:], vnT[:, s0:s0 + 128], w1t[:],
                                 start=False, stop=True)
                nc.scalar.copy(w1b2_hi[:, t, :], pw[:])
                nc.vector.tensor_tensor(w1b2_tmp[:], pw[:], w1b2_hi[:, t, :],
                                        op=mybir.AluOpType.subtract)
                nc.vector.tensor_copy(w1b2_lo[:, t, :], w1b2_tmp[:])

            sg_ps = sgp.tile([128, SEG_C], F32)
            zrow = const.tile([1, SEG_C], F32)
            nc.vector.memset(zrow[:], 0.0)
            nc.tensor.matmul(sg_ps[:], ones1[:], zrow[:],
                             start=True, stop=True, skip_group_check=True)

            for g in range(ng):
                x_sb = xs.tile([128, 4, H], F32)
                nc.sync.dma_start(
                    x_sb[:],
                    d["x"][g * 512:(g + 1) * 512, :].rearrange(
                        "(c p) h -> p c h", p=128))
                xT_sb = xs.tile([H, 512], F32)
                nc.sync.dma_start(xT_sb[:], d["xT"][:, g * 512:(g + 1) * 512])

                p1g = psw.tile([128, 512], F32, tag="p1")
                pmask = work.tile([128, 4, MW], BF16, tag="pm")
                for c in range(4):
                    n = g * 4 + c
                    st = windows[n]
                    nc.tensor.matmul(p1g[:, c * 128:(c + 1) * 128],
                                     xT_sb[:, c * 128:(c + 1) * 128],
                                     w2t[:], start=True, stop=False,
                                     skip_group_check=True)
                    nc.vector.tensor_scalar(
                        pmask[:, c, :], iota_f[:, st:st + MW],
                        blf[:, n:n + 1], None, mybir.AluOpType.is_equal)
                    tpm = pst.tile([MW, 128], BF16, tag="mt")
                    nc.tensor.transpose(tpm[:], pmask[:, c, :], ident_bf[:])
                    mT = work.tile([MW, 128], BF16, tag="mTs")
                    nc.scalar.copy(mT[:], tpm[:])
                    blk = {0: 0, 64: 2, 128: 1}[st]
                    nc.tensor.matmul(
                        p1g[:, c * 128:(c + 1) * 128], mT[:],
                        w1b2_hi[:, blk, :],
                        start=False, stop=False, skip_group_check=True)
                    nc.tensor.matmul(
                        p1g[:, c * 128:(c + 1) * 128], mT[:],
                        w1b2_lo[:, blk, :],
                        start=False, stop=True, skip_group_check=True)
                hsb = work.tile([128, 4, H], F32)
                nc.scalar.activation(hsb[:].rearrange("p a b -> p (a b)"),
                                     p1g[:],
                                     mybir.ActivationFunctionType.Sigmoid)
                hq = work.tile([128, 4, H], F32)
                nc.vector.tensor_tensor(hq[:], hsb[:], _bc(q_bcast[:], 1, 4),
                                        op=mybir.AluOpType.mult)
                araw = work.tile([128, 4], F32)
                nc.vector.reduce_sum(araw[:], hq[:], axis=mybir.AxisListType.X)
                alpha = work.tile([128, 4], F32)
                nc.vector.tensor_tensor(alpha[:], araw[:],
                                        qb[:].to_broadcast([128, 4]),
                                        op=mybir.AluOpType.add)
                mask = work.tile([128, 4, SW], F32, tag="ma")
                for c in range(4):
                    n = g * 4 + c
                    st = swin[n]
                    nc.vector.tensor_scalar(
                        mask[:, c, :], iota_f[:, st:st + SW],
                        blf[:, n:n + 1], alpha[:, c:c + 1],
                        mybir.AluOpType.is_equal, mybir.AluOpType.mult)
                    nc.tensor.matmul(
                        sg_ps[:, st:st + SW], x_sb[:, c, :], mask[:, c, :],
                        start=False, stop=(n == nt - 1),
                        skip_group_check=True)

            sgT = const.tile([H, SEG_C], F32)
            nc.vector.tensor_copy(sgT[:], sg_ps[:])
            shs = const.tile([128, 2, H], F32)
            for t in range(2):
                psh = pst.tile([128, 128], F32, tag="mt")
                nc.tensor.matmul(psh[:], ones1[:], w3brow[:], start=True,
                                 stop=False)
                nc.tensor.matmul(psh[:], vnT[:, t * 128:(t + 1) * 128],
                                 w3at[:], start=False, stop=False)
                nc.tensor.matmul(psh[:], sgT[:, t * 128:(t + 1) * 128],
                                 w3bt[:], start=False, stop=True)
                nc.vector.tensor_copy(shs[:, t, :], psh[:])
                nc.sync.dma_start(cc_in[t * 128:(t + 1) * 128, :],
                                  shs[:, t, :])

            # ---- all-gather s_h across the 8 cores ----
            nc.gpsimd.collective_compute(
                "AllGather", mybir.AluOpType.bypass,
                replica_groups=[list(range(8))],
                ins=[cc_in[:, :]], outs=[cc_out[:, :]])

            # ---- shT = gathered s_h transposed, split bf16 hi/lo ----
            shT = const.tile([H, B_SEG], F32)
            for a in range(16):
                gt = const.tile([128, 128], F32, tag="gt")
                nc.sync.dma_start(gt[:], cc_out[a * 128:(a + 1) * 128, :])
                gp = pst.tile([128, 128], F32, tag="mt")
                nc.tensor.transpose(gp[:], gt[:], ident[:])
                nc.vector.tensor_copy(shT[:, a * 128:(a + 1) * 128], gp[:])
            sh_hi = const.tile([H, B_SEG], BF16)
            nc.scalar.copy(sh_hi[:], shT[:])
            sh_tmp = const.tile([H, B_SEG], F32)
            nc.vector.tensor_tensor(sh_tmp[:], shT[:], sh_hi[:],
                                    op=mybir.AluOpType.subtract)
            sh_lo = const.tile([H, B_SEG], BF16)
            nc.vector.tensor_copy(sh_lo[:], sh_tmp[:])

            # ---- phase 2: z = s_h @ ET via bf16 hi/lo 3-matmul ----
            eth, etl = [], []
            for i in range(nch):
                a2 = const.tile([H, NCHUNK], BF16, tag=f"eth{i}")
                nc.sync.dma_start(a2[:], et_hi_d[:, i * NCHUNK:(i + 1) * NCHUNK])
                eth.append(a2)
                b2 = const.tile([H, NCHUNK], BF16, tag=f"etl{i}")
                nc.sync.dma_start(b2[:], et_lo_d[:, i * NCHUNK:(i + 1) * NCHUNK])
                etl.append(b2)
            kk = 0
            for m in range(B_SEG // 128):
                ms = slice(m * 128, (m + 1) * 128)
                for i in range(nch):
                    stg = stage.tile([128, NCHUNK], F32)
                    for j in range(ntm):
                        js = slice(j * NTILE, (j + 1) * NTILE)
                        pz = pst.tile([128, NTILE], F32, tag="mt")
                        nc.tensor.matmul(pz[:], sh_hi[:, ms], eth[i][:, js],
                                         start=True, stop=False)
                        nc.tensor.matmul(pz[:], sh_hi[:, ms], etl[i][:, js],
                                         start=False, stop=False)
                        nc.tensor.matmul(pz[:], sh_lo[:, ms], eth[i][:, js],
                                         start=False, stop=True)
                        dst = stg[:, js]
                        if kk % 2 == 0:
                            nc.vector.tensor_copy(dst, pz[:])
                        else:
                            nc.scalar.copy(dst, pz[:])
                        kk += 1
                    nc.sync.dma_start(
                        z_d[m * 128:(m + 1) * 128,
                            i * NCHUNK:(i + 1) * NCHUNK], stg[:])
    nc.compile()
    return nc


def _split_hilo(a):
    hi = a.astype(ml_dtypes.bfloat16)
    lo = (a - hi.astype(np.float32)).astype(ml_dtypes.bfloat16)
    return hi, lo


def _prep(inputs):
    """Host-side: shard inputs, derive index tensors from `batch`."""
    batch = np.asarray(inputs["batch"]).astype(np.int64)
    x = np.ascontiguousarray(np.asarray(inputs["session_embedding"], np.float32))
    emb = np.ascontiguousarray(np.asarray(inputs["emb_weight"], np.float32))

    starts = np.searchsorted(batch, np.arange(0, B_SEG + 1, SEG_C))
    counts = np.diff(starts)
    nmax = int(-(-counts.max() // 512) * 512)

    last_idx = np.searchsorted(batch, np.arange(B_SEG) + 1) - 1  # [B]

    w1t = np.ascontiguousarray(np.asarray(inputs["W1_w"], np.float32).T)
    w2t = np.ascontiguousarray(np.asarray(inputs["W2_w"], np.float32).T)
    w3 = np.asarray(inputs["W3_w"], np.float32)
    w3at = np.ascontiguousarray(w3[:, :H].T)
    w3bt = np.ascontiguousarray(w3[:, H:].T)
    b12 = (np.asarray(inputs["W1_b"], np.float32)
           + np.asarray(inputs["W2_b"], np.float32)).reshape(1, H)
    w3brow = np.asarray(inputs["W3_b"], np.float32).reshape(1, H)
    qrow = np.asarray(inputs["q_w"], np.float32).reshape(1, H)
    qb = np.asarray(inputs["q_b"], np.float32).reshape(1, 1)

    in1, in2 = [], []
    nt = nmax // 128
    for c in range(NCORES):
        st, en = int(starts[c]), int(starts[c + 1])
        cnt = en - st
        xc = np.zeros((nmax, H), np.float32)
        xc[:cnt] = x[st:en]
        blc = np.full(nmax, SEG_C - 1, np.int64)
        blc[:cnt] = batch[st:en] - c * SEG_C
        lastl = (last_idx[c * SEG_C:(c + 1) * SEG_C] - st).astype(np.int32)
        in1.append({
            "x": xc,
            "xT": np.ascontiguousarray(xc.T),
            "blf": np.ascontiguousarray(
                blc.reshape(nt, 128).T.astype(np.float32)),
            "bli": np.ascontiguousarray(
                blc.reshape(nt, 128).T.astype(np.int32)),
            "lastloc": np.ascontiguousarray(lastl.reshape(2, 128).T),
            "W1T": w1t, "W2T": w2t, "W3aT": w3at, "W3bT": w3bt,
            "b12": b12, "w3brow": w3brow, "qrow": qrow, "qb": qb,
        })
        v0 = 1 + c * VSHARD
        v1 = min(v0 + VSHARD, VOCAB)
        etc = np.zeros((VSHARD, H), np.float32)
        etc[:v1 - v0] = emb[v0:v1]
        et_hi, et_lo = _split_hilo(np.ascontiguousarray(etc.T))
        in2.append({"ET_hi": et_hi, "ET_lo": et_lo})

    windows = affine_windows(nmax, MW, grid=64)
    swin = affine_windows(nmax, SW)
    blfs = [m["blf"] for m in in1]
    use_windows = (windows_ok(blfs, nmax, MW, windows)
                   and windows_ok(blfs, nmax, SW, swin))
    return in1, in2, nmax, use_windows


_CACHE = {}


def _get_programs(nmax, use_windows=True):
    key = (nmax, use_windows)
    if key not in _CACHE:
        if use_windows:
            nc1 = _build_phase1(nmax, affine_windows(nmax, MW, grid=64),
                                affine_windows(nmax, SW))
        else:
            nc1 = _build_phase1_fallback(nmax)
        _CACHE[key] = (nc1, _build_phase2())
    return _CACHE[key]


def _get_merged(nmax):
    key = ("merged", nmax)
    if key not in _CACHE:
        _CACHE[key] = _build_merged(nmax, affine_windows(nmax, MW, grid=64),
                                    affine_windows(nmax, SW))
    return _CACHE[key]


def kernel(**inputs) -> np.ndarray:
    in1, in2, nmax, use_windows = _prep(inputs)

    if use_windows:
        # single launch: phase1 + on-device AllGather of s_h + phase2
        nc = _get_merged(nmax)
        ins = []
        for c in range(NCORES):
            m = {k: v for k, v in in1[c].items() if k != "bli"}
            m.update(in2[c])
            ins.append(m)
        res = bass_utils.run_bass_kernel_spmd(nc, ins,
                                              core_ids=list(range(NCORES)))
        z = np.concatenate([res.results[c]["z"] for c in range(NCORES)], axis=1)
        return np.ascontiguousarray(z[:, :VOCAB - 1])

    # fallback: two launches with host gather of s_h
    nc1, nc2 = _get_programs(nmax, use_windows=False)
    res1 = bass_utils.run_bass_kernel_spmd(nc1, in1, core_ids=list(range(NCORES)))
    sh = np.concatenate([res1.results[c]["s_h"] for c in range(NCORES)], axis=0)
    sh_hi, sh_lo = _split_hilo(np.ascontiguousarray(sh.T))  # [H, B_SEG]
    for m in in2:
        m["shT_hi"] = sh_hi
        m["shT_lo"] = sh_lo
    res2 = bass_utils.run_bass_kernel_spmd(nc2, in2, core_ids=list(range(NCORES)))
    z = np.concatenate([res2.results[c]["z"] for c in range(NCORES)], axis=1)
    return np.ascontiguousarray(z[:, :VOCAB - 1])

